# revision 18
# baseline (speedup 1.0000x reference)
"""Trainium2 Bass kernel for the 3-block binarized MLP (BNN). V4.

V4 structural changes over V3:
  - th1 (layer-1 BN threshold = batch mean of z1) is computed on HOST:
    mean_b(z1) = W1b @ sum_b(x) / B depends only on the inputs, so it is
    precomputable like the weight binarization. This removes the layer-1
    collective entirely AND lets u1 / layer-2 matmuls stream INSIDE the
    layer-1 loop (no barrier).
  - Layer-2/3 stats: sum_b(c) = W @ sum_b(u) commutes through the matmul.
    DVE reduces over the u tiles feed FD=1 fp32 matmuls, so each
    AllReduce launches before the c copies finish (latency mostly
    hidden), and the c copies carry no accumulation burden.
  - u1 is computed by DVE directly from PSUM (z1 is never materialized;
    the host-provided th1 makes layer-1 stats unnecessary).
  - Everything else (hi/lo host split, {0,1} activations, f16 z storage
    with swapped-operand is_le, contiguous L5, natural-layout output)
    as in V3.
"""

import os
import numpy as np

import concourse.bacc as bacc
import concourse.mybir as mybir
import concourse.tile as tile
from concourse.bass_utils import run_bass_kernel_spmd

N_CORES = 8
B = 65536
BS = B // N_CORES          # 8192 rows per core
D_IN = 1024
H1, H2, H3, D_OUT = 200, 100, 100, 10
NCH = 8                    # batch chunks per core
CH = BS // NCH             # 1024
KC = D_IN // 128           # 8 k-chunks
NQ = 4                     # DMA quarters per chunk (k-pairs)

f32 = mybir.dt.float32
f16 = mybir.dt.float16
AF = mybir.ActivationFunctionType
ALU = mybir.AluOpType

REP_BARRIER = 1
HLN = 2


def build_nc(reps=1, single=False):
    ndev = 1 if single else N_CORES
    nc = bacc.Bacc(
        "TRN2", target_bir_lowering=False, debug=False, num_devices=ndev
    )
    xhl = nc.declare_dram_parameter("xhl", [NCH * 128, KC * 2 * CH], f16,
                                    isOutput=False)
    wb1 = nc.declare_dram_parameter("wb1", [128, KC * H1], f16, isOutput=False)
    wb2 = nc.declare_dram_parameter("wb2", [100, 2 * H2], f16, isOutput=False)
    wb3 = nc.declare_dram_parameter("wb3", [100, H3], f16, isOutput=False)
    wb5 = nc.declare_dram_parameter("wb5", [100, D_OUT], f16, isOutput=False)
    b5p = nc.declare_dram_parameter("b5p", [128, D_OUT], f32, isOutput=False)
    th1d = nc.declare_dram_parameter("th1d", [100, 2], f32, isOutput=False)
    y = nc.declare_dram_parameter("y", [128, NCH * 8 * D_OUT], f32,
                                  isOutput=True)

    groups = [list(range(N_CORES))]

    def _allreduce(spool, dpool, sums_sb, ncols, tagpfx):
        ar_in = dpool.tile([100, ncols], f32, tag=f"{tagpfx}i",
                           name=f"{tagpfx}i")
        ar_out = dpool.tile([100, ncols], f32, tag=f"{tagpfx}o",
                            name=f"{tagpfx}o")
        nc.sync.dma_start(ar_in[:], sums_sb)
        if single:
            nc.sync.dma_start(ar_out[:], ar_in[:])
        else:
            nc.gpsimd.collective_compute(
                "AllReduce", ALU.add, replica_groups=groups,
                ins=[ar_in.opt()], outs=[ar_out.opt()],
            )
        gs = spool.tile([100, ncols], f32, tag=f"{tagpfx}g", name=f"{tagpfx}g")
        nc.sync.dma_start(gs[:], ar_out[:])
        return gs

    with tile.TileContext(nc) as tc:
        with (
            tc.tile_pool(name="consts", bufs=1) as cpool,
            tc.tile_pool(name="xq", bufs=5) as xpool,
            tc.tile_pool(name="zc", bufs=2) as zcpool,
            tc.tile_pool(name="u", bufs=3) as upool,
            tc.tile_pool(name="small", bufs=1) as spool,
            tc.tile_pool(name="psum", bufs=3, space="PSUM") as ppool,
            tc.tile_pool(name="psum5", bufs=2, space="PSUM") as ppool5,
            tc.tile_pool(name="dram", bufs=1, space="DRAM") as dpool,
        ):
            # ---- constants (outside rep loop) ---------------------------
            w1s = cpool.tile([128, KC, H1], f16, tag="w1s")
            nc.sync.dma_start(w1s[:], wb1[:, :])
            w2s = cpool.tile([100, 2, H2], f16, tag="w2s")
            nc.sync.dma_start(w2s[:], wb2[:, :])
            w3s = cpool.tile([100, H3], f16, tag="w3s")
            nc.sync.dma_start(w3s[:], wb3[:, :])
            w5s = cpool.tile([100, D_OUT], f16, tag="w5s")
            nc.sync.dma_start(w5s[:], wb5[:, :])
            b5s = cpool.tile([128, D_OUT], f32, tag="b5s")
            nc.sync.dma_start(b5s[:], b5p[:, :])
            th1 = cpool.tile([100, 2], f32, tag="th1")
            nc.sync.dma_start(th1[:], th1d[:, :])
            # fp32 copies of w2/w3 for the exact FD=1 stat matmuls
            w2f = cpool.tile([100, 2, H2], f32, tag="w2f")
            nc.scalar.copy(w2f[:], w2s[:])
            w3f = cpool.tile([100, H3], f32, tag="w3f")
            nc.scalar.copy(w3f[:], w3s[:])

            xhl_r = xhl.rearrange("(n p) f -> p n f", p=128)

            if reps > 1:
                tc.strict_bb_all_engine_barrier()
            for _rep in range(reps):
                if _rep and REP_BARRIER:
                    tc.strict_bb_all_engine_barrier()

                # ---- fused layer 1 + layer 2 stream ---------------------
                su1 = spool.tile([100, 2 * NCH], f32, tag="su1")
                z2h = [zcpool.tile([100, NCH // 2, CH], f16, tag="zc",
                                   name=f"z2h{h}") for h in range(2)]
                th1_b = [th1[:, m:m + 1].broadcast_to((100, CH))
                         for m in range(2)]
                u1tiles = {}

                def emit_l2(n):
                    u1 = u1tiles.pop(n)
                    ps2 = ppool.tile([100, CH], f32, tag="ps",
                                     name=f"ps2_{n}")
                    for c in range(2):
                        for g in range(2):
                            gsl = slice(g * 512, (g + 1) * 512)
                            nc.tensor.matmul(ps2[:, gsl], w2s[:, c, :],
                                             u1[:, c, gsl],
                                             start=(c == 0), stop=(c == 1))
                    zdst = z2h[n // (NCH // 2)][:, n % (NCH // 2), :]
                    nc.scalar.activation(zdst, ps2[:], AF.Copy)

                for n in range(NCH):
                    ps = [ppool.tile([100, CH], f32, tag="ps",
                                     name=f"ps1_{n}_{m}") for m in range(2)]
                    for q in range(NQ):
                        xq = xpool.tile([128, 2, 2, CH], f16)
                        nc.sync.dma_start(
                            xq[:].rearrange("p a b c -> p (a b c)"),
                            xhl_r[:, n, q * 2 * 2 * CH:(q + 1) * 2 * 2 * CH])
                        for kk in range(2):
                            k = q * 2 + kk
                            for m in range(2):
                                wk = w1s[:, k, m * 100:(m + 1) * 100]
                                for hl in range(HLN):
                                    for g in range(2):
                                        gsl = slice(g * 512, (g + 1) * 512)
                                        nc.tensor.matmul(
                                            ps[m][:, gsl], wk,
                                            xq[:, kk, hl, gsl],
                                            start=(k == 0 and hl == 0),
                                            stop=(k == KC - 1
                                                  and hl == HLN - 1))
                    u1 = upool.tile([100, 2, CH], f16, tag="u1")
                    for m in range(2):
                        nc.vector.tensor_tensor(
                            u1[:, m, :], th1_b[m], ps[m][:], ALU.is_le)
                    nc.vector.tensor_reduce(
                        su1[:, 2 * n:2 * n + 2],
                        u1[:, :, :], mybir.AxisListType.X, ALU.add)
                    u1tiles[n] = u1
                    if n >= 1:
                        emit_l2(n - 1)
                emit_l2(NCH - 1)

                # ---- layer-2 stats: sum_b c2 = W2b @ sum_b u1 -----------
                su1g = spool.tile([100, 2], f32, tag="su1g")
                nc.vector.tensor_reduce(
                    su1g[:], su1[:, :].rearrange("p (n m) -> p m n", m=2),
                    mybir.AxisListType.X, ALU.add)
                st2ps = ppool5.tile([100, 1], f32, tag="ps5", name="st2ps")
                for c in range(2):
                    nc.tensor.matmul(st2ps[:], w2f[:, c, :], su1g[:, c:c + 1],
                                     start=(c == 0), stop=(c == 1))
                s2 = spool.tile([100, 1], f32, tag="s2")
                nc.vector.tensor_scalar(s2[:], st2ps[:], 1.0, None, ALU.mult)
                gs2 = _allreduce(spool, dpool, s2[:], 1, "ar2")

                # ---- layer 3 --------------------------------------------
                z3h = [zcpool.tile([100, NCH // 2, CH], f16, tag="zc",
                                   name=f"z3h{h}") for h in range(2)]
                su2 = spool.tile([100, NCH], f32, tag="su2")
                su2_defer = []

                def emit_su2(n, u2):
                    nc.vector.tensor_reduce(
                        su2[:, n:n + 1],
                        u2[:, :].rearrange("p (o f) -> p o f", o=1),
                        mybir.AxisListType.X, ALU.add)

                s2_b = gs2[:, 0:1].broadcast_to((100, CH))
                for n in range(NCH):
                    u2 = upool.tile([100, CH], f16, tag="u2")
                    zsrc1 = z2h[n // (NCH // 2)][:, n % (NCH // 2), :]
                    nc.vector.scalar_tensor_tensor(
                        u2[:], zsrc1, float(B), s2_b, ALU.mult, ALU.is_ge)
                    if su2_defer:
                        emit_su2(*su2_defer.pop())
                    su2_defer.append((n, u2))
                    ps = ppool.tile([100, CH], f32, tag="ps",
                                    name=f"ps3_{n}")
                    for g in range(2):
                        gsl = slice(g * 512, (g + 1) * 512)
                        nc.tensor.matmul(ps[:, gsl], w3s[:],
                                         u2[:, gsl],
                                         start=True, stop=True)
                    zdst = z3h[n // (NCH // 2)][:, n % (NCH // 2), :]
                    if n % 4 == 3:
                        nc.vector.tensor_scalar(zdst, ps[:], 1.0, None,
                                                ALU.mult)
                    else:
                        nc.scalar.activation(zdst, ps[:], AF.Copy)

                # ---- layer-3 stats: sum_b c3 = W3b @ sum_b u2 -----------
                if su2_defer:
                    emit_su2(*su2_defer.pop())
                su2g = spool.tile([100, 1], f32, tag="su2g")
                nc.vector.tensor_reduce(
                    su2g[:], su2[:, :].rearrange("p (o n) -> p o n", o=1),
                    mybir.AxisListType.X, ALU.add)
                st3ps = ppool5.tile([100, 1], f32, tag="ps5", name="st3ps")
                nc.tensor.matmul(st3ps[:], w3f[:], su2g[:],
                                 start=True, stop=True)
                s3 = spool.tile([100, 1], f32, tag="s3")
                nc.vector.tensor_scalar(s3[:], st3ps[:], 1.0, None, ALU.mult)
                gs3 = _allreduce(spool, dpool, s3[:], 1, "ar3")

                # warm the Exp/Ln ACT tables while the PE runs layer 5
                wa = spool.tile([128, 1], f32, tag="wa")
                wb = spool.tile([128, 1], f32, tag="wb")
                nc.vector.memset(wa[:], 0.0)
                nc.scalar.activation(wb[:], wa[:], AF.Exp)
                nc.scalar.activation(wa[:], wb[:], AF.Ln)

                # ---- layer 5 + log_softmax ------------------------------
                # batch row = n*CH + j*128 + p
                y5 = spool.tile([128, NCH, 8, D_OUT], f32, tag="y5")
                b5_b = b5s[:, :].rearrange("p (x c) -> p x c", x=1) \
                                .broadcast_to((128, 8, D_OUT))
                s3_b = gs3[:, 0:1].rearrange("p (a x) -> p a x", a=1) \
                    .broadcast_to((100, 8, 128))
                u3tiles = {}

                def emit_u3(n):
                    u3 = upool.tile([100, 8, 128], f16, tag="u3")
                    zsrc = z3h[n // (NCH // 2)][:, n % (NCH // 2), :]
                    nc.vector.scalar_tensor_tensor(
                        u3[:, :, :],
                        zsrc.rearrange("k (j p) -> k j p", p=128),
                        float(B), s3_b, ALU.mult, ALU.is_ge)
                    u3tiles[n] = u3

                emit_u3(0)
                for n in range(NCH):
                    if n + 1 < NCH:
                        emit_u3(n + 1)
                    u3 = u3tiles.pop(n)
                    ps5 = ppool5.tile([128, 8, D_OUT], f32,
                                      tag="ps5", name=f"ps5_{n}")
                    for j in range(8):
                        nc.tensor.matmul(ps5[:, j, :], u3[:, j, :], w5s[:],
                                         start=True, stop=True)
                    nc.vector.scalar_tensor_tensor(
                        y5[:, n, :, :], ps5[:], 2.0, b5_b,
                        ALU.mult, ALU.add)

                # log_softmax in two halves (pipeline scalar/vector)
                for h in range(2):
                    nsl = slice(h * (NCH // 2), (h + 1) * (NCH // 2))
                    yg = y5[:, nsl, :, :].rearrange("p n j c -> p (n j) c")
                    nj = (NCH // 2) * 8
                    mx = spool.tile([128, nj], f32, tag=f"mx{h}")
                    nc.vector.tensor_reduce(mx[:], yg, mybir.AxisListType.X,
                                            ALU.max)
                    mx_b = mx[:, :].rearrange("p (a x) -> p a x", x=1) \
                                   .broadcast_to((128, nj, D_OUT))
                    zm = spool.tile([128, nj, D_OUT], f32, tag=f"zm{h}")
                    nc.vector.tensor_tensor(zm[:], yg, mx_b, ALU.subtract)
                    ex = spool.tile([128, nj, D_OUT], f32, tag=f"ex{h}")
                    se = spool.tile([128, nj], f32, tag=f"se{h}")
                    exf = ex[:, :, :].rearrange("p a c -> p (a c)")
                    nc.scalar.activation(exf, zm[:].rearrange(
                        "p a c -> p (a c)"), AF.Exp)
                    nc.vector.tensor_reduce(se[:], ex[:], mybir.AxisListType.X,
                                            ALU.add)
                    ln = spool.tile([128, nj], f32, tag=f"ln{h}")
                    nc.scalar.activation(ln[:], se[:], AF.Ln)
                    ln_b = ln[:, :].rearrange("p (a x) -> p a x", x=1) \
                                   .broadcast_to((128, nj, D_OUT))
                    yo = spool.tile([128, NCH // 2, 8, D_OUT], f32,
                                    tag=f"yo{h}")
                    yo_f = yo[:, :, :, :].rearrange("p n j c -> p (n j) c")
                    nc.vector.tensor_tensor(yo_f, zm[:], ln_b, ALU.subtract)
                    nc.sync.dma_start(
                        y.rearrange("p (n j c) -> p n j c", n=NCH,
                                    j=8)[:, nsl, :, :], yo[:])

    nc.compile()
    return nc


_NC_CACHE = None


def _get_nc():
    global _NC_CACHE
    if _NC_CACHE is None:
        _NC_CACHE = build_nc()
    return _NC_CACHE


def prepare_in_maps(x, w1, w2, w3, w5, b5, g1, be1, g2, be2, g3, be3):
    """Host-side sharding / layout prep -> one input map per core."""
    x = np.asarray(x, dtype=np.float32)
    w1 = np.asarray(w1, dtype=np.float32)
    w2 = np.asarray(w2, dtype=np.float32)
    w3 = np.asarray(w3, dtype=np.float32)
    w5 = np.asarray(w5, dtype=np.float32)
    b5 = np.asarray(b5, dtype=np.float32)
    # The kernel exploits beta==0 / gamma>0 (threshold == batch mean).
    for g, be in ((g1, be1), (g2, be2), (g3, be3)):
        assert np.all(np.asarray(be) == 0.0), "kernel assumes beta == 0"
        assert np.all(np.asarray(g) > 0.0), "kernel assumes gamma > 0"

    def binz(w):
        return np.where(w >= 0, np.float16(1.0), np.float16(-1.0))

    w1b = binz(w1)                                    # [200, 1024]
    wb1 = np.ascontiguousarray(
        w1b.T.reshape(KC, 128, H1).transpose(1, 0, 2)).reshape(128, KC * H1)
    w2b = binz(w2)                                    # [100, 200]
    wb2 = np.ascontiguousarray(
        w2b.T.reshape(2, 100, H2).transpose(1, 0, 2)).reshape(100, 2 * H2)
    wb3 = np.ascontiguousarray(binz(w3).T)            # [100, 100]
    w5b = binz(w5)                                    # [10, 100]
    wb5 = np.ascontiguousarray(w5b.T)                 # [100, 10]
    b5f = (b5 - w5b.astype(np.float32).sum(axis=1)).astype(np.float32)
    b5p = np.ascontiguousarray(np.tile(b5f[None, :], (128, 1)))

    # layer-1 threshold = batch mean of z1 = W1b @ sum_b(x) / B
    sx = x.astype(np.float64).sum(axis=0)             # [1024]
    th1v = (w1b.astype(np.float64) @ sx) / float(B)   # [200]
    th1 = np.ascontiguousarray(
        th1v.reshape(2, 100).T.astype(np.float32))    # [100, 2]

    in_maps = []
    for c in range(N_CORES):
        xsT = x[c * BS:(c + 1) * BS].T                # [1024, 8192]
        hi = xsT.astype(np.float16)
        lo = (xsT - hi.astype(np.float32)).astype(np.float16)
        H = hi.reshape(KC, 128, NCH, CH)
        L = lo.reshape(KC, 128, NCH, CH)
        X = np.stack([H, L], axis=3)                  # [k, p, n, hl, b]
        X = np.ascontiguousarray(X.transpose(2, 1, 0, 3, 4))  # [n,p,k,hl,b]
        in_maps.append({
            "xhl": X.reshape(NCH * 128, KC * 2 * CH),
            "wb1": wb1, "wb2": wb2, "wb3": wb3, "wb5": wb5, "b5p": b5p,
            "th1d": th1,
        })
    return in_maps


def kernel(x, w1, b1, g1, be1, w2, b2, g2, be2, w3, b3, g3, be3, w5, b5):
    # b1/b2/b3 cancel inside training-mode BatchNorm and are unused.
    nc = _get_nc()
    in_maps = prepare_in_maps(
        np.asarray(x), np.asarray(w1), np.asarray(w2), np.asarray(w3),
        np.asarray(w5), np.asarray(b5), np.asarray(g1), np.asarray(be1),
        np.asarray(g2), np.asarray(be2), np.asarray(g3), np.asarray(be3),
    )
    res = run_bass_kernel_spmd(nc, in_maps, core_ids=list(range(N_CORES)))
    outs = []
    for c in range(N_CORES):
        ysb = res.results[c]["y"].reshape(128, NCH, 8, D_OUT)
        # batch row = n*CH + j*128 + p
        outs.append(np.ascontiguousarray(
            ysb.transpose(1, 2, 0, 3)).reshape(BS, D_OUT))
    return np.concatenate(outs, axis=0)
